# revision 15
# baseline (speedup 1.0000x reference)
"""Trainium2 Bass kernel for ClassLinearWithLORA (moe_routing).

Computes out = x @ W.T + b + gates[-1] * (alpha * (x @ A[-1]) @ B_lora[-1])
(the torch loop overwrites out_lora each class iteration, so only the last
class adapter contributes).

Strategy (v2 — fp8 DoubleRow):
  - Data-parallel shard of the 8192 (B*S) rows across 8 NeuronCores
    (1024 rows/core); W/b and the rank-16 LoRA stacks are replicated.
  - All matmuls run in fp8e4 (e4m3) with MatmulPerfMode.DoubleRow: one
    instruction contracts 2x128 K-values at 0.5 cycles/row — 4x the
    fp32r contraction rate per instruction.
  - Precision is recovered with a compensated split: x = x_h + x_l and
    W' = 64*W = W_h + W_l (both parts fp8). The main GEMM runs 3 passes
    (x_h@W_h + x_h@W_l + x_l@W_h); the dropped x_l@W_l term is O(eps^2).
    Per output tile that is 12 DoubleRow matmuls (3072 cycles) vs 8
    fp32r matmuls (4096 cycles).
  - The LoRA first matmul (x@A) uses the same 3-pass split, with A's
    columns duplicated to 32 so the psum rows 16-31 carry a copy used
    to build the l_l part lane-aligned on the Vector engine.
  - The rank-16 LoRA update + bias fold into ONE DoubleRow aug matmul
    per tile (K = 2x32): subtile0 = [l_h; l_l] x [B_h; B_h], subtile1 =
    [l_h; ones; ones; 0...] x [B_l; b_h; b_l; 0...], where
    l = 16*g*(x@A) is built on-device in fp8 h+l parts.
  - Everything accumulates in one PSUM bank at scale S=64; the epilogue
    is a tensor_scalar_mul by 1/64 on the Vector engine writing fp16
    tiles, stored per-tile on the ACT HWDGE ring (last tile split
    across both rings to shorten the tail).

Measured: relative error ~2.2e-3 vs the fp32 jax reference (harness
gate 2e-2); per-core cost-model PE floor for this decomposition is
~91 us vs ~126 us for the fp32r formulation.
"""

import numpy as np
import ml_dtypes

import concourse.bacc as bacc
import concourse.mybir as mybir
import concourse.tile as tile
from concourse.bass_utils import run_bass_kernel_spmd

F32 = mybir.dt.float32
F16 = mybir.dt.float16
FP8 = mybir.dt.float8e4
DR = mybir.MatmulPerfMode.DoubleRow
NP8 = ml_dtypes.float8_e4m3

N_CORES = 8
B, S, D_IN, D_OUT = 4, 2048, 1024, 4096
R_LORA = 16
ROWS = B * S                  # 8192
R_CORE = ROWS // N_CORES      # 1024 rows per core
KT2 = D_IN // 256             # 4 double-K chunks (each 2x128)
NB = 512                      # moving free dim per matmul
OB = D_OUT // NB              # 8 output blocks
RT = R_CORE // 128            # 8 row tiles per core
SCALE = 64.0                  # PSUM carries 64x the true output


def _build():
    nc = bacc.Bacc(None, target_bir_lowering=False)

    xh_d = nc.dram_tensor("xh", [128, KT2, 2, R_CORE], FP8, kind="ExternalInput")
    xl_d = nc.dram_tensor("xl", [128, KT2, 2, R_CORE], FP8, kind="ExternalInput")
    wh_d = nc.dram_tensor("wh", [128, OB, KT2, 2, NB], FP8, kind="ExternalInput")
    wl_d = nc.dram_tensor("wl", [128, OB, KT2, 2, NB], FP8, kind="ExternalInput")
    ah_d = nc.dram_tensor("ah", [128, KT2, 2, 32], FP8, kind="ExternalInput")
    al_d = nc.dram_tensor("al", [128, KT2, 2, 32], FP8, kind="ExternalInput")
    rhs_d = nc.dram_tensor("rhs_aug", [32, 2, D_OUT], FP8, kind="ExternalInput")
    g_d = nc.dram_tensor("g2", [32, R_CORE], F32, kind="ExternalInput")
    msk_d = nc.dram_tensor("mask", [32, R_CORE], FP8, kind="ExternalInput")
    # rows 0-1 = ones (laug sub1 lanes 16-17), rows 2-15 = zeros (lanes 18-31)
    ac_d = nc.dram_tensor("aug_const", [16, R_CORE], FP8, kind="ExternalInput")
    # output laid out [p, rt, ob, n] so half-ob store groups are single DMAs
    out_d = nc.dram_tensor("out", [128, RT, OB, NB], F16, kind="ExternalOutput")

    with tile.TileContext(nc) as tc:
        with (
            tc.tile_pool(name="resident", bufs=1) as res,
            tc.tile_pool(name="wpool", bufs=5) as wpool,
            tc.tile_pool(name="opool", bufs=4) as opool,
            tc.tile_pool(name="psum", bufs=8, space="PSUM") as psum,
        ):
            # ---- resident loads -------------------------------------------
            # SP ring: wt block 0 halves first (first matmuls unblock early),
            # then the aug constants, then steady-state weight prefetch.
            # ACT ring: A stacks + x halves (later the output stores). DMAs
            # are batched: every HWDGE issue costs ~630ns on a single shared
            # descriptor-generator, so fewer, bigger transfers win.
            wh0 = wpool.tile([128, KT2, 2, NB], FP8, tag="wt", name="wh0")
            nc.sync.dma_start(wh0[:, 0:2], wh_d.ap()[:, 0, 0:2])
            ah = res.tile([128, KT2, 2, 32], FP8)
            nc.scalar.dma_start(ah[:], ah_d.ap())
            xh = res.tile([128, KT2, 2, R_CORE], FP8)
            nc.scalar.dma_start(xh[:, 0:2], xh_d.ap()[:, 0:2])
            nc.sync.dma_start(wh0[:, 2:4], wh_d.ap()[:, 0, 2:4])
            nc.scalar.dma_start(xh[:, 2:4], xh_d.ap()[:, 2:4])
            wl0 = wpool.tile([128, KT2, 2, NB], FP8, tag="wt", name="wl0")
            nc.sync.dma_start(wl0[:, 0:2], wl_d.ap()[:, 0, 0:2])
            nc.sync.dma_start(wl0[:, 2:4], wl_d.ap()[:, 0, 2:4])
            al = res.tile([128, KT2, 2, 32], FP8)
            nc.scalar.dma_start(al[:], al_d.ap())
            xl = res.tile([128, KT2, 2, R_CORE], FP8)
            nc.scalar.dma_start(xl[:, 0:2], xl_d.ap()[:, 0:2])
            nc.scalar.dma_start(xl[:, 2:4], xl_d.ap()[:, 2:4])

            g2 = res.tile([32, R_CORE], F32)
            nc.sync.dma_start(g2[:], g_d.ap())
            msk = res.tile([32, R_CORE], FP8)
            nc.sync.dma_start(msk[:], msk_d.ap())
            rhs_aug = res.tile([32, 2, D_OUT], FP8)
            nc.sync.dma_start(rhs_aug[:], rhs_d.ap())
            laug = res.tile([32, 2, R_CORE], FP8)
            nc.sync.dma_start(laug[16:32, 1], ac_d.ap())
            v_sb = res.tile([32, R_CORE], F32)
            tmph = res.tile([32, R_CORE], FP8)
            m_sb = res.tile([32, R_CORE], FP8)

            # prefetch ob=1 weights during the prologue
            wts = {0: (wh0, wl0)}

            def load_wt(ob):
                whb = wpool.tile([128, KT2, 2, NB], FP8, tag="wt", name=f"wh{ob}")
                nc.sync.dma_start(whb[:], wh_d.ap()[:, ob])
                wlb = wpool.tile([128, KT2, 2, NB], FP8, tag="wt", name=f"wl{ob}")
                nc.sync.dma_start(wlb[:], wl_d.ap()[:, ob])
                wts[ob] = (whb, wlb)

            load_wt(1)

            # ---- prologue: ob=0 mains + LoRA, paced by chunk arrivals -----
            ps_l = [psum.tile([32, NB], F32, tag="ps", name=f"psl{rb}") for rb in range(2)]
            ps0 = [psum.tile([128, NB], F32, tag="ps", name=f"ps0_{rt}") for rt in range(6)]

            def lora_mm(xt_sb, a_sb, t, first, last):
                for rb in range(2):
                    nc.tensor.matmul(
                        ps_l[rb][:],
                        a_sb[:, t],
                        xt_sb[:, t, :, rb * NB : (rb + 1) * NB],
                        start=first, stop=last, perf_mode=DR,
                    )

            def main_mm(ps, xt_sb, wt_sb, t, rt, first, last=False):
                nc.tensor.matmul(
                    ps[:],
                    xt_sb[:, t, :, rt * 128 : (rt + 1) * 128],
                    wt_sb[:, t],
                    start=first, stop=last, perf_mode=DR,
                )

            stages = ((xh, wh0, ah), (xh, wl0, al), (xl, wh0, ah))
            for si, (xt_sb, wt_sb, a_sb) in enumerate(stages):
                for t in range(KT2):
                    lora_mm(xt_sb, a_sb, t,
                            first=(si == 0 and t == 0),
                            last=(si == 2 and t == KT2 - 1))
                    for rt in range(6):
                        main_mm(ps0[rt], xt_sb, wt_sb, t, rt,
                                first=(si == 0 and t == 0))

            # gated LoRA intermediate -> fp8 h+l parts. Engine SBUF accesses
            # must start at a 32-aligned partition, so sub0 = [l_h; l_l] is
            # produced by full 32-lane ops: psum rows 16-31 duplicate rows
            # 0-15 (duplicated A columns); mask is 0 on lanes 0-15 and 1 on
            # lanes 16-31, so fp8(v - fp8(v)*mask) = [l_h; l_l] in one op.
            for rb in range(2):
                sl = slice(rb * NB, (rb + 1) * NB)
                nc.vector.tensor_mul(v_sb[:, sl], ps_l[rb][:], g2[:, sl])
            nc.vector.tensor_copy(tmph[:], v_sb[:])
            nc.vector.tensor_mul(m_sb[:], tmph[:], msk[:])
            nc.vector.tensor_sub(laug[:, 0], v_sb[:], m_sb[:])
            nc.vector.tensor_copy(laug[0:16, 1], v_sb[0:16])

            # rt 6,7 mains keep the PE busy while the DVE builds laug
            ps67 = {}
            for rt in (6, 7):
                ps = psum.tile([128, NB], F32, tag="ps", name=f"ps0b{rt}")
                first = True
                for xt_sb, wt_sb, _ in stages:
                    for t in range(KT2):
                        main_mm(ps, xt_sb, wt_sb, t, rt, first=first)
                        first = False
                ps67[rt] = ps

            def emit_aug(ps, rt, ob, start, stop, cols=slice(0, NB)):
                nc.tensor.matmul(
                    ps[:],
                    laug[:, :, rt * 128 : (rt + 1) * 128],
                    rhs_aug[:, :, ob * NB + cols.start : ob * NB + cols.stop],
                    start=start, stop=stop, perf_mode=DR,
                )

            # Epilogue: scale-copy each psum tile into a 4-row staging group
            # on the DVE; one store DMA per group (HWDGE issues are costly).
            ogroups = {}

            def emit_epilogue(ps, rt, ob):
                g, i = rt // 4, rt % 4
                if i == 0:
                    ogroups[(ob, g)] = opool.tile(
                        [128, 4, NB], F16, tag="o", name=f"o{ob}_{g}"
                    )
                o_sb = ogroups[(ob, g)]
                nc.vector.tensor_scalar_mul(o_sb[:, i], ps[:], 1.0 / SCALE)
                if i == 3:
                    nc.scalar.dma_start(
                        out_d.ap()[:, g * 4 : (g + 1) * 4, ob], o_sb[:]
                    )

            for rt in range(6):
                emit_aug(ps0[rt], rt, 0, start=False, stop=True)
                emit_epilogue(ps0[rt], rt, 0)
            for rt in (6, 7):
                emit_aug(ps67[rt], rt, 0, start=False, stop=True)
                emit_epilogue(ps67[rt], rt, 0)

            # ---- steady state: ob = 1..7 ----------------------------------
            for ob in range(1, OB):
                if ob + 1 < OB:
                    load_wt(ob + 1)
                whb, wlb = wts[ob]
                last_ob = ob == OB - 1
                for rt in range(RT):
                    if last_ob and rt == RT - 1:
                        break
                    ps = psum.tile([128, NB], F32, tag="ps", name=f"ps{ob}_{rt}")
                    emit_aug(ps, rt, ob, start=True, stop=False)
                    for si, (xt_sb, wt_sb) in enumerate(
                        ((xh, whb), (xh, wlb), (xl, whb))
                    ):
                        for t in range(KT2):
                            main_mm(ps, xt_sb, wt_sb, t, rt, first=False,
                                    last=(si == 2 and t == KT2 - 1))
                    emit_epilogue(ps, rt, ob)

            # ---- tail: ob=7 rt=7 in two half-width psum chains so the first
            # half's epilogue+store overlap the second half's matmuls -------
            ob, rt = OB - 1, RT - 1
            whb, wlb = wts[ob]
            o_sb = ogroups[(ob, 1)]
            # rt 4-6 were staged at rows 0-2 by emit_epilogue; store them now
            nc.scalar.dma_start(out_d.ap()[:, 4:7, ob], o_sb[:, 0:3])
            H = NB // 2
            for hf, ring in ((0, nc.scalar), (1, nc.sync)):
                cols = slice(hf * H, (hf + 1) * H)
                ps = psum.tile([128, H], F32, tag="ps", name=f"ps_tail{hf}")
                emit_aug(ps, rt, ob, start=True, stop=False, cols=cols)
                for si, (xt_sb, wt_sb) in enumerate(
                    ((xh, whb), (xh, wlb), (xl, whb))
                ):
                    for t in range(KT2):
                        nc.tensor.matmul(
                            ps[:],
                            xt_sb[:, t, :, rt * 128 : (rt + 1) * 128],
                            wt_sb[:, t, :, cols],
                            start=False,
                            stop=(si == 2 and t == KT2 - 1),
                            perf_mode=DR,
                        )
                nc.vector.tensor_scalar_mul(o_sb[:, 3, cols], ps[:], 1.0 / SCALE)
                ring.dma_start(out_d.ap()[:, rt, ob, cols], o_sb[:, 3, cols])

    nc.compile()
    return nc


_NC_CACHE = None


def _get_nc():
    global _NC_CACHE
    if _NC_CACHE is None:
        _NC_CACHE = _build()
    return _NC_CACHE


def _hi_lo(a):
    """fp32 array -> (high fp8, low fp8) with a + err = high + low + O(eps^2)."""
    h = np.ascontiguousarray(a, dtype=np.float32).astype(NP8)
    l = (a - h.astype(np.float32)).astype(NP8)
    return h, l


def _prep_in_maps(x, W, b, A, B_lora, gates, alpha):
    x = np.asarray(x, dtype=np.float32).reshape(ROWS, D_IN)
    W = np.asarray(W, dtype=np.float32)
    b = np.asarray(b, dtype=np.float32)
    A1 = np.asarray(A, dtype=np.float32)[-1]          # [D_IN, 16]
    B1 = np.asarray(B_lora, dtype=np.float32)[-1]     # [16, D_OUT]
    g = np.asarray(gates, dtype=np.float32)[-1].reshape(ROWS)
    alpha_f = float(np.asarray(alpha))

    # W' = 64*W packed [ki, ob, t, i, n] with k = t*256 + i*128 + ki
    wh, wl = _hi_lo((SCALE * W).astype(np.float32).T)          # [K, O]

    def pack_w(wq):
        return np.ascontiguousarray(
            wq.reshape(KT2, 2, 128, OB, NB).transpose(2, 3, 0, 1, 4))

    wh_p, wl_p = pack_w(wh), pack_w(wl)

    # A' = 64*A with columns duplicated to 32 (psum rows 16-31 = copy)
    ahq, alq = _hi_lo((SCALE * A1).astype(np.float32))         # [K, 16]

    def pack_a(aq):
        a32 = np.concatenate([aq, aq], axis=1)                 # [K, 32]
        return np.ascontiguousarray(
            a32.reshape(KT2, 2, 128, 32).transpose(2, 0, 1, 3))

    ah_p, al_p = pack_a(ahq), pack_a(alq)

    # aug rhs: sub0 = [B_h; B_h], sub1 = [B_l; b_h; b_l; 0...]
    # B' = 4*alpha*B so that (16*g*xA) @ B' = 64 * g*alpha*(xA@B)
    bh8, bl8 = _hi_lo((SCALE / 16.0 * alpha_f * B1).astype(np.float32))
    bbh, bbl = _hi_lo((SCALE * b).astype(np.float32))
    rhs = np.zeros((32, 2, D_OUT), dtype=NP8)
    rhs[0:16, 0] = bh8
    rhs[16:32, 0] = bh8
    rhs[0:16, 1] = bl8
    rhs[16, 1] = bbh
    rhs[17, 1] = bbl

    aug_const = np.zeros((16, R_CORE), dtype=NP8)
    aug_const[0:2] = 1.0
    mask = np.zeros((32, R_CORE), dtype=NP8)
    mask[16:32] = 1.0

    def pack_x(xq):
        return np.ascontiguousarray(
            xq.reshape(KT2, 2, 128, R_CORE).transpose(2, 0, 1, 3))

    in_maps = []
    for c in range(N_CORES):
        rows = slice(c * R_CORE, (c + 1) * R_CORE)
        xhq, xlq = _hi_lo(x[rows].T)                           # [K, R_CORE]
        g2 = np.ascontiguousarray(
            np.broadcast_to((0.25 * g[rows])[None, :], (32, R_CORE))
        ).astype(np.float32)
        in_maps.append(
            {
                "xh": pack_x(xhq),
                "xl": pack_x(xlq),
                "wh": wh_p,
                "wl": wl_p,
                "ah": ah_p,
                "al": al_p,
                "rhs_aug": rhs,
                "g2": g2,
                "mask": mask,
                "aug_const": aug_const,
            }
        )
    return in_maps


def run(inputs: dict, trace: bool = False, trace_cores=None):
    """Run the kernel; returns (full_output, BassKernelResults)."""
    nc = _get_nc()
    in_maps = _prep_in_maps(**inputs)
    res = run_bass_kernel_spmd(
        nc,
        in_maps,
        core_ids=list(range(N_CORES)),
        trace=trace,
        trace_cores=trace_cores,
    )
    # out is [p, rt, ob, n]; true row = rt*128 + p, col = ob*NB + n
    out = np.concatenate(
        [
            np.asarray(r["out"])
            .astype(np.float32)
            .transpose(1, 0, 2, 3)
            .reshape(R_CORE, D_OUT)
            for r in res.results
        ],
        axis=0,
    )
    return out.reshape(B, S, D_OUT), res


def kernel(**inputs) -> np.ndarray:
    out, _ = run(inputs, trace=False)
    return out


# revision 18
# speedup vs baseline: 1.0078x; 1.0078x over previous
"""Trainium2 Bass kernel for ClassLinearWithLORA (moe_routing).

Computes out = x @ W.T + b + gates[-1] * (alpha * (x @ A[-1]) @ B_lora[-1])
(the torch loop overwrites out_lora each class iteration, so only the last
class adapter contributes).

Strategy (v2 — fp8 DoubleRow):
  - Data-parallel shard of the 8192 (B*S) rows across 8 NeuronCores
    (1024 rows/core); W/b and the rank-16 LoRA stacks are replicated.
  - All matmuls run in fp8e4 (e4m3) with MatmulPerfMode.DoubleRow: one
    instruction contracts 2x128 K-values at 0.5 cycles/row — 4x the
    fp32r contraction rate per instruction.
  - Precision is recovered with a compensated split: x = x_h + x_l and
    W' = 64*W = W_h + W_l (both parts fp8). The main GEMM runs 3 passes
    (x_h@W_h + x_h@W_l + x_l@W_h); the dropped x_l@W_l term is O(eps^2).
    Per output tile that is 12 DoubleRow matmuls (3072 cycles) vs 8
    fp32r matmuls (4096 cycles).
  - The LoRA first matmul (x@A) uses the same 3-pass split, with A's
    columns duplicated to 32 so the psum rows 16-31 carry a copy used
    to build the l_l part lane-aligned on the Vector engine.
  - The rank-16 LoRA update + bias fold into ONE DoubleRow aug matmul
    per tile (K = 2x32): subtile0 = [l_h; l_l] x [B_h; B_h], subtile1 =
    [l_h; ones; ones; 0...] x [B_l; b_h; b_l; 0...], where
    l = 16*g*(x@A) is built on-device in fp8 h+l parts.
  - Everything accumulates in one PSUM bank at scale S=64; the epilogue
    is a tensor_scalar_mul by 1/64 on the Vector engine writing fp16
    tiles, stored per-tile on the ACT HWDGE ring (last tile split
    across both rings to shorten the tail).

Measured: relative error ~2.2e-3 vs the fp32 jax reference (harness
gate 2e-2); per-core cost-model PE floor for this decomposition is
~91 us vs ~126 us for the fp32r formulation.
"""

import numpy as np
import ml_dtypes

import concourse.bacc as bacc
import concourse.mybir as mybir
import concourse.tile as tile
from concourse.bass_utils import run_bass_kernel_spmd

F32 = mybir.dt.float32
F16 = mybir.dt.float16
FP8 = mybir.dt.float8e4
DR = mybir.MatmulPerfMode.DoubleRow
NP8 = ml_dtypes.float8_e4m3

N_CORES = 8
B, S, D_IN, D_OUT = 4, 2048, 1024, 4096
R_LORA = 16
ROWS = B * S                  # 8192
R_CORE = ROWS // N_CORES      # 1024 rows per core
KT2 = D_IN // 256             # 4 double-K chunks (each 2x128)
NB = 512                      # moving free dim per matmul
OB = D_OUT // NB              # 8 output blocks
RT = R_CORE // 128            # 8 row tiles per core
SCALE = 64.0                  # PSUM carries 64x the true output


def _build():
    nc = bacc.Bacc(None, target_bir_lowering=False)

    xh_d = nc.dram_tensor("xh", [128, KT2, 2, R_CORE], FP8, kind="ExternalInput")
    xl_d = nc.dram_tensor("xl", [128, KT2, 2, R_CORE], FP8, kind="ExternalInput")
    wh_d = nc.dram_tensor("wh", [128, OB, KT2, 2, NB], FP8, kind="ExternalInput")
    wl_d = nc.dram_tensor("wl", [128, OB, KT2, 2, NB], FP8, kind="ExternalInput")
    ah_d = nc.dram_tensor("ah", [128, KT2, 2, 32], FP8, kind="ExternalInput")
    al_d = nc.dram_tensor("al", [128, KT2, 2, 32], FP8, kind="ExternalInput")
    rhs_d = nc.dram_tensor("rhs_aug", [32, 2, D_OUT], FP8, kind="ExternalInput")
    g_d = nc.dram_tensor("g2", [32, R_CORE], F32, kind="ExternalInput")
    msk_d = nc.dram_tensor("mask", [32, R_CORE], FP8, kind="ExternalInput")
    # rows 0-1 = ones (laug sub1 lanes 16-17), rows 2-15 = zeros (lanes 18-31)
    ac_d = nc.dram_tensor("aug_const", [16, R_CORE], FP8, kind="ExternalInput")
    # output laid out [p, rt, ob, n] so half-ob store groups are single DMAs
    out_d = nc.dram_tensor("out", [128, RT, OB, NB], F16, kind="ExternalOutput")

    with tile.TileContext(nc) as tc:
        with (
            tc.tile_pool(name="resident", bufs=1) as res,
            tc.tile_pool(name="wpool", bufs=5) as wpool,
            tc.tile_pool(name="opool", bufs=4) as opool,
            tc.tile_pool(name="psum", bufs=8, space="PSUM") as psum,
        ):
            # ---- resident loads -------------------------------------------
            # SP ring: wt block 0 halves first (first matmuls unblock early),
            # then the aug constants, then steady-state weight prefetch.
            # ACT ring: A stacks + x halves (later the output stores). DMAs
            # are batched: every HWDGE issue costs ~630ns on a single shared
            # descriptor-generator, so fewer, bigger transfers win.
            # first chunks load individually (transfer latency to chunk 0
            # gates the first matmul; DMA transfers serialize globally)
            ah = res.tile([128, KT2, 2, 32], FP8)
            nc.scalar.dma_start(ah[:], ah_d.ap())
            wh0 = wpool.tile([128, KT2, 2, NB], FP8, tag="wt", name="wh0")
            xh = res.tile([128, KT2, 2, R_CORE], FP8)
            for t in range(KT2):
                nc.scalar.dma_start(xh[:, t], xh_d.ap()[:, t])
                nc.sync.dma_start(wh0[:, t], wh_d.ap()[:, 0, t])
            al = res.tile([128, KT2, 2, 32], FP8)
            nc.scalar.dma_start(al[:], al_d.ap())
            wl0 = wpool.tile([128, KT2, 2, NB], FP8, tag="wt", name="wl0")
            xl = res.tile([128, KT2, 2, R_CORE], FP8)
            for t in range(KT2):
                nc.scalar.dma_start(xl[:, t], xl_d.ap()[:, t])
                nc.sync.dma_start(wl0[:, t], wl_d.ap()[:, 0, t])

            g2 = res.tile([32, R_CORE], F32)
            nc.sync.dma_start(g2[:], g_d.ap())
            msk = res.tile([32, R_CORE], FP8)
            nc.sync.dma_start(msk[:], msk_d.ap())
            rhs_aug = res.tile([32, 2, D_OUT], FP8)
            nc.sync.dma_start(rhs_aug[:], rhs_d.ap())
            laug = res.tile([32, 2, R_CORE], FP8)
            nc.sync.dma_start(laug[16:32, 1], ac_d.ap())
            v_sb = res.tile([32, R_CORE], F32)
            tmph = res.tile([32, R_CORE], FP8)
            m_sb = res.tile([32, R_CORE], FP8)

            # prefetch ob=1 weights during the prologue
            wts = {0: (wh0, wl0)}

            def load_wt(ob):
                whb = wpool.tile([128, KT2, 2, NB], FP8, tag="wt", name=f"wh{ob}")
                nc.sync.dma_start(whb[:], wh_d.ap()[:, ob])
                wlb = wpool.tile([128, KT2, 2, NB], FP8, tag="wt", name=f"wl{ob}")
                nc.sync.dma_start(wlb[:], wl_d.ap()[:, ob])
                wts[ob] = (whb, wlb)

            load_wt(1)

            # ---- prologue: ob=0 mains + LoRA, paced by chunk arrivals -----
            ps_l = [psum.tile([32, NB], F32, tag="ps", name=f"psl{rb}") for rb in range(2)]
            ps0 = [psum.tile([128, NB], F32, tag="ps", name=f"ps0_{rt}") for rt in range(6)]

            def lora_mm(xt_sb, a_sb, t, first, last):
                for rb in range(2):
                    nc.tensor.matmul(
                        ps_l[rb][:],
                        a_sb[:, t],
                        xt_sb[:, t, :, rb * NB : (rb + 1) * NB],
                        start=first, stop=last, perf_mode=DR,
                    )

            def main_mm(ps, xt_sb, wt_sb, t, rt, first, last=False):
                nc.tensor.matmul(
                    ps[:],
                    xt_sb[:, t, :, rt * 128 : (rt + 1) * 128],
                    wt_sb[:, t],
                    start=first, stop=last, perf_mode=DR,
                )

            stages = ((xh, wh0, ah), (xh, wl0, al), (xl, wh0, ah))
            for si, (xt_sb, wt_sb, a_sb) in enumerate(stages):
                for t in range(KT2):
                    lora_mm(xt_sb, a_sb, t,
                            first=(si == 0 and t == 0),
                            last=(si == 2 and t == KT2 - 1))
                    for rt in range(6):
                        main_mm(ps0[rt], xt_sb, wt_sb, t, rt,
                                first=(si == 0 and t == 0))

            # gated LoRA intermediate -> fp8 h+l parts. Engine SBUF accesses
            # must start at a 32-aligned partition, so sub0 = [l_h; l_l] is
            # produced by full 32-lane ops: psum rows 16-31 duplicate rows
            # 0-15 (duplicated A columns); mask is 0 on lanes 0-15 and 1 on
            # lanes 16-31, so fp8(v - fp8(v)*mask) = [l_h; l_l] in one op.
            # Built in column halves: the rt 0-3 aug matmuls only need the
            # first 512 columns, halving the latency to the first aug.
            for rb in range(2):
                sl = slice(rb * NB, (rb + 1) * NB)
                nc.vector.tensor_mul(v_sb[:, sl], ps_l[rb][:], g2[:, sl])
                nc.vector.tensor_copy(tmph[:, sl], v_sb[:, sl])
                nc.vector.tensor_mul(m_sb[:, sl], tmph[:, sl], msk[:, sl])
                nc.vector.tensor_sub(laug[:, 0, sl], v_sb[:, sl], m_sb[:, sl])
                nc.vector.tensor_copy(laug[0:16, 1, sl], v_sb[0:16, sl])

            # rt 6,7 mains keep the PE busy while the DVE builds laug
            ps67 = {}
            for rt in (6, 7):
                ps = psum.tile([128, NB], F32, tag="ps", name=f"ps0b{rt}")
                first = True
                for xt_sb, wt_sb, _ in stages:
                    for t in range(KT2):
                        main_mm(ps, xt_sb, wt_sb, t, rt, first=first)
                        first = False
                ps67[rt] = ps

            def emit_aug(ps, rt, ob, start, stop, cols=slice(0, NB)):
                nc.tensor.matmul(
                    ps[:],
                    laug[:, :, rt * 128 : (rt + 1) * 128],
                    rhs_aug[:, :, ob * NB + cols.start : ob * NB + cols.stop],
                    start=start, stop=stop, perf_mode=DR,
                )

            # Epilogue: scale-copy each psum tile into a 4-row staging group
            # on the DVE; one store DMA per group (HWDGE issues are costly).
            ogroups = {}

            def emit_epilogue(ps, rt, ob):
                g, i = rt // 4, rt % 4
                if i == 0:
                    ogroups[(ob, g)] = opool.tile(
                        [128, 4, NB], F16, tag="o", name=f"o{ob}_{g}"
                    )
                o_sb = ogroups[(ob, g)]
                nc.vector.tensor_scalar_mul(o_sb[:, i], ps[:], 1.0 / SCALE)
                if i == 3:
                    nc.scalar.dma_start(
                        out_d.ap()[:, g * 4 : (g + 1) * 4, ob], o_sb[:]
                    )

            for rt in range(6):
                emit_aug(ps0[rt], rt, 0, start=False, stop=True)
                emit_epilogue(ps0[rt], rt, 0)
            for rt in (6, 7):
                emit_aug(ps67[rt], rt, 0, start=False, stop=True)
                emit_epilogue(ps67[rt], rt, 0)

            # ---- steady state: ob = 1..7 ----------------------------------
            for ob in range(1, OB):
                if ob + 1 < OB:
                    load_wt(ob + 1)
                whb, wlb = wts[ob]
                last_ob = ob == OB - 1
                for rt in range(RT):
                    if last_ob and rt == RT - 1:
                        break
                    ps = psum.tile([128, NB], F32, tag="ps", name=f"ps{ob}_{rt}")
                    emit_aug(ps, rt, ob, start=True, stop=False)
                    for si, (xt_sb, wt_sb) in enumerate(
                        ((xh, whb), (xh, wlb), (xl, whb))
                    ):
                        for t in range(KT2):
                            main_mm(ps, xt_sb, wt_sb, t, rt, first=False,
                                    last=(si == 2 and t == KT2 - 1))
                    emit_epilogue(ps, rt, ob)

            # ---- tail: ob=7 rt=7 in two half-width psum chains so the first
            # half's epilogue+store overlap the second half's matmuls -------
            ob, rt = OB - 1, RT - 1
            whb, wlb = wts[ob]
            o_sb = ogroups[(ob, 1)]
            # rt 4-6 were staged at rows 0-2 by emit_epilogue; store them now
            nc.scalar.dma_start(out_d.ap()[:, 4:7, ob], o_sb[:, 0:3])
            H = NB // 4
            for hf, ring in ((0, nc.scalar), (1, nc.sync), (2, nc.scalar), (3, nc.sync)):
                cols = slice(hf * H, (hf + 1) * H)
                ps = psum.tile([128, H], F32, tag="ps", name=f"ps_tail{hf}")
                emit_aug(ps, rt, ob, start=True, stop=False, cols=cols)
                for si, (xt_sb, wt_sb) in enumerate(
                    ((xh, whb), (xh, wlb), (xl, whb))
                ):
                    for t in range(KT2):
                        nc.tensor.matmul(
                            ps[:],
                            xt_sb[:, t, :, rt * 128 : (rt + 1) * 128],
                            wt_sb[:, t, :, cols],
                            start=False,
                            stop=(si == 2 and t == KT2 - 1),
                            perf_mode=DR,
                        )
                nc.vector.tensor_scalar_mul(o_sb[:, 3, cols], ps[:], 1.0 / SCALE)
                ring.dma_start(out_d.ap()[:, rt, ob, cols], o_sb[:, 3, cols])

    nc.compile()
    return nc


_NC_CACHE = None


def _get_nc():
    global _NC_CACHE
    if _NC_CACHE is None:
        _NC_CACHE = _build()
    return _NC_CACHE


def _hi_lo(a):
    """fp32 array -> (high fp8, low fp8) with a + err = high + low + O(eps^2)."""
    h = np.ascontiguousarray(a, dtype=np.float32).astype(NP8)
    l = (a - h.astype(np.float32)).astype(NP8)
    return h, l


def _prep_in_maps(x, W, b, A, B_lora, gates, alpha):
    x = np.asarray(x, dtype=np.float32).reshape(ROWS, D_IN)
    W = np.asarray(W, dtype=np.float32)
    b = np.asarray(b, dtype=np.float32)
    A1 = np.asarray(A, dtype=np.float32)[-1]          # [D_IN, 16]
    B1 = np.asarray(B_lora, dtype=np.float32)[-1]     # [16, D_OUT]
    g = np.asarray(gates, dtype=np.float32)[-1].reshape(ROWS)
    alpha_f = float(np.asarray(alpha))

    # W' = 64*W packed [ki, ob, t, i, n] with k = t*256 + i*128 + ki
    wh, wl = _hi_lo((SCALE * W).astype(np.float32).T)          # [K, O]

    def pack_w(wq):
        return np.ascontiguousarray(
            wq.reshape(KT2, 2, 128, OB, NB).transpose(2, 3, 0, 1, 4))

    wh_p, wl_p = pack_w(wh), pack_w(wl)

    # A' = 64*A with columns duplicated to 32 (psum rows 16-31 = copy)
    ahq, alq = _hi_lo((SCALE * A1).astype(np.float32))         # [K, 16]

    def pack_a(aq):
        a32 = np.concatenate([aq, aq], axis=1)                 # [K, 32]
        return np.ascontiguousarray(
            a32.reshape(KT2, 2, 128, 32).transpose(2, 0, 1, 3))

    ah_p, al_p = pack_a(ahq), pack_a(alq)

    # aug rhs: sub0 = [B_h; B_h], sub1 = [B_l; b_h; b_l; 0...]
    # B' = 4*alpha*B so that (16*g*xA) @ B' = 64 * g*alpha*(xA@B)
    bh8, bl8 = _hi_lo((SCALE / 16.0 * alpha_f * B1).astype(np.float32))
    bbh, bbl = _hi_lo((SCALE * b).astype(np.float32))
    rhs = np.zeros((32, 2, D_OUT), dtype=NP8)
    rhs[0:16, 0] = bh8
    rhs[16:32, 0] = bh8
    rhs[0:16, 1] = bl8
    rhs[16, 1] = bbh
    rhs[17, 1] = bbl

    aug_const = np.zeros((16, R_CORE), dtype=NP8)
    aug_const[0:2] = 1.0
    mask = np.zeros((32, R_CORE), dtype=NP8)
    mask[16:32] = 1.0

    def pack_x(xq):
        return np.ascontiguousarray(
            xq.reshape(KT2, 2, 128, R_CORE).transpose(2, 0, 1, 3))

    in_maps = []
    for c in range(N_CORES):
        rows = slice(c * R_CORE, (c + 1) * R_CORE)
        xhq, xlq = _hi_lo(x[rows].T)                           # [K, R_CORE]
        g2 = np.ascontiguousarray(
            np.broadcast_to((0.25 * g[rows])[None, :], (32, R_CORE))
        ).astype(np.float32)
        in_maps.append(
            {
                "xh": pack_x(xhq),
                "xl": pack_x(xlq),
                "wh": wh_p,
                "wl": wl_p,
                "ah": ah_p,
                "al": al_p,
                "rhs_aug": rhs,
                "g2": g2,
                "mask": mask,
                "aug_const": aug_const,
            }
        )
    return in_maps


def run(inputs: dict, trace: bool = False, trace_cores=None):
    """Run the kernel; returns (full_output, BassKernelResults)."""
    nc = _get_nc()
    in_maps = _prep_in_maps(**inputs)
    res = run_bass_kernel_spmd(
        nc,
        in_maps,
        core_ids=list(range(N_CORES)),
        trace=trace,
        trace_cores=trace_cores,
    )
    # out is [p, rt, ob, n]; true row = rt*128 + p, col = ob*NB + n
    out = np.concatenate(
        [
            np.asarray(r["out"])
            .astype(np.float32)
            .transpose(1, 0, 2, 3)
            .reshape(R_CORE, D_OUT)
            for r in res.results
        ],
        axis=0,
    )
    return out.reshape(B, S, D_OUT), res


def kernel(**inputs) -> np.ndarray:
    out, _ = run(inputs, trace=False)
    return out
